# revision 1
# baseline (speedup 1.0000x reference)
"""MemoryBank scatter-gather kernel for 8 Trainium2 NeuronCores.

Reference (per token n of 2048, K=500 neighbor slots padded with index 0):
    neigh = l2norm(wordmem[idx[n]]); q = l2norm(word_embs[n])
    score = q @ neigh.T
    attn  = exp(score) * (k < len) / sum(...)
    out   = attn @ bankmem[idx[n]]

Strategy (v2, "token-major everywhere"):
  * Tokens are split on the host into "pieces" of <= K0 neighbor rows
    (per-bank round-robin so piece bank-counts stay balanced).  Pieces are
    sorted by max per-bank count and packed into NBLK slots of 1024 pieces
    (8 cores x 128 partitions), so the shared per-slot gather budgets
    BW[s][b] = max piece count are tight.  Softmax normalization is
    deferred: each piece returns an UNNORMALIZED weighted sum (400 f32)
    plus its exp-sum denominator; the host merges pieces of a token and
    divides once.  This makes splitting exact (no approximation).
  * ONE fused bf16 table in HBM: row = [l2norm(wordmem) 100 | bankmem 400
    | pad] = 512 elems = 1024B (>= 512B, so full-rate DMA descriptors).
    A single token-major gather per chunk feeds BOTH the score and the
    output path: partition = piece, column = neighbor j, bank-segmented
    with per-slot budgets.
  * Per chunk of MC columns: DVE bf16 mult (q broadcast) + reduce ->
    dots, + additive NEG mask slice, ACT Exp -> exps (bf16, per-chunk f32
    accumulators summed into the piece denominator), then for column j,
    matmul psum[128,400] += W_j^T @ fused[:,j,100:500], alternating two
    PSUM accumulation chains (even/odd chunks) for pipeline slack.
    W_j = diag(exps[:, j]) is built per chunk on DVE as identX * exps
    (identX = identity replicated along a packed last dim, keeping all
    operands in the DVE 2x/4x fast path).
"""

import numpy as np
import ml_dtypes

import concourse.bacc as bacc
import concourse.bass as bass
import concourse.mybir as mybir
import concourse.tile as tile
from concourse.bass_utils import run_bass_kernel_spmd
from concourse.masks import make_identity

BF16 = ml_dtypes.bfloat16

NUM = 2048
K = 500
WD = 100
HD = 400
V = 100000
N_CORES = 8
NBANK = 4
BROWS = V // NBANK             # 25000 rows per bank (< int16 max)
FDP = 512                      # fused row elems: 100 w-hat + 400 bank + pad
                               # (512 bf16 = 1024B, full-rate descriptors)
NBLK = 6                       # slots = blocks of 128 pieces per core
K0 = 101                       # max neighbor rows per piece
MC = 16                        # fused-gather chunk (columns)
NPS = 3                        # parallel PSUM accumulation chains
NEG = -1.0e9
DOCW = HD + 1                  # 400 outputs + 1 denominator

_CACHE: dict = {}


# --------------------------------------------------------------------------
# host planning
# --------------------------------------------------------------------------

def _plan(idx, lengths):
    """Split tokens into pieces, pack pieces into slots, compute budgets."""
    idx = np.asarray(idx, dtype=np.int64)
    lengths = np.asarray(lengths, dtype=np.int64)

    # pieces: (token, [per-bank int16 local index lists]); bump K0 until
    # the piece count fits the slot capacity (robust to other length draws)
    cap = NBLK * 1024
    k0 = K0
    while sum(-(-int(le) // k0) for le in lengths) > cap:
        k0 += 4
    pieces = []
    for t in range(NUM):
        v = idx[t, : lengths[t]]
        b = v // BROWS
        lists = [(v[b == bb] - bb * BROWS).astype(np.int16)
                 for bb in range(NBANK)]
        p = -(-int(lengths[t]) // k0)
        for i in range(p):
            pieces.append((t, [lb[i::p] for lb in lists]))
    while len(pieces) < cap:
        pieces.append((-1, [np.zeros(0, np.int16)] * NBANK))

    cnt = np.array([[len(lb) for lb in ls] for _t, ls in pieces])
    order = np.argsort(-cnt.max(axis=1), kind="stable")

    slots = []
    for s in range(NBLK):
        ranks = order[s * 1024 : (s + 1) * 1024]
        BW = cnt[ranks].max(axis=0)                      # [NBANK]
        SW = np.concatenate([[0], np.cumsum(BW)])
        slots.append({"BW": BW.astype(int), "SW": SW.astype(int),
                      "KPW": int(SW[-1]),
                      # ranks laid out [core, partition]
                      "ranks": ranks.reshape(8, 128)})
    return {"pieces": pieces, "slots": slots}


def _wrap16(flat_i16):
    """i-th index -> (i%16, i//16), replicated to 128 partitions."""
    n = flat_i16.shape[0]
    assert n % 16 == 0
    blk = flat_i16.reshape(-1, 16).T.copy()            # [16, n/16]
    return np.tile(blk, (8, 1))                        # [128, n/16]


def _per_core_arrays(plan, we_hat16, core):
    out = {}
    for s, sl in enumerate(plan["slots"]):
        KPW, SW, BW = sl["KPW"], sl["SW"], sl["BW"]
        ranks = sl["ranks"][core]                      # [128] piece ids

        q2 = np.zeros((128, WD), dtype=BF16)
        ix = np.zeros((128, KPW), dtype=np.int16)
        mneg = np.full((128, KPW), NEG, dtype=BF16)
        for p, r in enumerate(ranks):
            t, lists = plan["pieces"][r]
            if t < 0:
                continue
            q2[p, :WD] = we_hat16[t]
            for b in range(NBANK):
                c = len(lists[b])
                ix[p, SW[b] : SW[b] + c] = lists[b]
                mneg[p, SW[b] : SW[b] + c] = 0.0
        ix16 = (np.concatenate(
            [_wrap16(ix[:, SW[b] : SW[b + 1]].T.ravel()) for b in range(NBANK)
             if BW[b] > 0], axis=1)
            if KPW else np.zeros((128, 0), np.int16))
        out[f"q{s}"] = q2
        out[f"mneg{s}"] = mneg
        out[f"ix{s}"] = ix16
    return out


# --------------------------------------------------------------------------
# bass program (built per budget signature)
# --------------------------------------------------------------------------

def _build_nc(plan):
    nc = bacc.Bacc(None, target_bir_lowering=False)
    bf = mybir.dt.bfloat16
    f32 = mybir.dt.float32

    tb_d = nc.dram_tensor("tb", [V, FDP], bf, kind="ExternalInput")
    doc_d = nc.dram_tensor("doc", [NBLK * 128, DOCW], f32,
                           kind="ExternalOutput")
    slot_in = []
    for s, sl in enumerate(plan["slots"]):
        KPW = max(sl["KPW"], 1)
        slot_in.append({
            "q": nc.dram_tensor(f"q{s}", [128, WD], bf, kind="ExternalInput"),
            "mneg": nc.dram_tensor(f"mneg{s}", [128, KPW], bf,
                                   kind="ExternalInput"),
            "ix": nc.dram_tensor(f"ix{s}", [128, 8 * KPW], mybir.dt.int16,
                                 kind="ExternalInput"),
        })

    KPW_MX = max(max(sl["KPW"] for sl in plan["slots"]), 1)
    NCH_MX = max(sum(-(-int(c) // MC) for c in sl["BW"])
                 for sl in plan["slots"])

    with tile.TileContext(nc) as tc:
        with (
            tc.tile_pool(name="const", bufs=1) as const,
            tc.tile_pool(name="per_blk", bufs=3) as per_blk,
            tc.tile_pool(name="gpool", bufs=8) as gpool,
            tc.tile_pool(name="ppool", bufs=6) as ppool,
            tc.tile_pool(name="wxpool", bufs=6) as wxpool,
            tc.tile_pool(name="small", bufs=8) as small,
            tc.tile_pool(name="psum_o", bufs=2, space="PSUM") as psum_o_pool,
        ):
            ident = const.tile([128, 128], bf)
            make_identity(nc, ident[:])
            # identity replicated along a PACKED last dim of MC so the
            # W-build tensor_tensor keeps every operand 2-byte/stride-1
            identx = const.tile([128, 128, MC], bf)
            nc.vector.tensor_copy(
                out=identx[:],
                in_=ident[:, :, None].to_broadcast([128, 128, MC]),
            )

            for s, sl in enumerate(plan["slots"]):
                KPW, SW, BW = sl["KPW"], sl["SW"], sl["BW"]
                if KPW == 0:
                    continue
                din = slot_in[s]

                q_t = per_blk.tile([128, WD], bf, tag="q_t")
                nc.sync.dma_start(out=q_t[:], in_=din["q"][:, :])
                mneg_t = per_blk.tile([128, KPW_MX], bf, tag="mneg_t")
                nc.sync.dma_start(out=mneg_t[:, :KPW], in_=din["mneg"][:, :])
                # whole slot's gather indices in one upload (keeps the
                # per-chunk dependency chain one stage shorter)
                ix_t = per_blk.tile([128, 8 * KPW_MX], mybir.dt.int16,
                                    tag="ix_t")
                nc.sync.dma_start(out=ix_t[:, : 8 * KPW], in_=din["ix"][:, :])

                # NPS independent PSUM accumulation chains give the
                # per-chunk score chain slack to run ahead of the matmuls
                pss = [psum_o_pool.tile([128, DOCW], f32, tag=f"psum_{par}",
                                        name=f"ps{par}_{s}")
                       for par in range(NPS)]
                chunks = []
                for b in range(NBANK):
                    if BW[b] == 0:
                        continue
                    for j0 in range(0, int(BW[b]), MC):
                        chunks.append((b, j0, min(MC, int(BW[b]) - j0)))
                nch = len(chunks)
                nps = min(NPS, nch)
                first = [True] * nps
                last_ci = [max(i for i in range(nch) if i % nps == par)
                           for par in range(nps)]
                for ci, (b, j0, cols) in enumerate(chunks):
                    gcol = int(SW[b]) + j0
                    n = 128 * cols
                    g_t = gpool.tile([128, MC, FDP], bf, tag="g_t")
                    nc.gpsimd.dma_gather(
                        out_ap=g_t[:, :cols, :],
                        in_ap=tb_d[b * BROWS : (b + 1) * BROWS, :],
                        idxs_ap=ix_t[:, 8 * gcol : 8 * (gcol + cols)],
                        num_idxs=n, num_idxs_reg=n, elem_size=FDP,
                        single_packet=False,
                    )
                    # score: dots = sum_d(w * q) (100 real dims), mask, exp
                    prod = ppool.tile([128, MC, WD], bf, tag="prod")
                    nc.vector.tensor_tensor(
                        out=prod[:, :cols, :], in0=g_t[:, :cols, :WD],
                        in1=q_t[:, None, :].to_broadcast([128, cols, WD]),
                        op=mybir.AluOpType.mult,
                    )
                    dots = small.tile([128, MC], bf, tag="dots")
                    with nc.allow_low_precision(
                        reason="bf16 cosine scores; gate is 2e-2"
                    ):
                        nc.vector.tensor_reduce(
                            out=dots[:, :cols],
                            in_=prod[:, :cols, :],
                            axis=mybir.AxisListType.X,
                            op=mybir.AluOpType.add,
                        )
                    nc.vector.tensor_tensor(
                        out=dots[:, :cols], in0=dots[:, :cols],
                        in1=mneg_t[:, gcol : gcol + cols],
                        op=mybir.AluOpType.add,
                    )
                    exps = small.tile([128, MC], bf, tag="exps")
                    nc.scalar.activation(
                        out=exps[:, :cols], in_=dots[:, :cols],
                        func=mybir.ActivationFunctionType.Exp,
                    )
                    # output: psum += diag(exps[:, jj]) @ bank rows
                    wx = wxpool.tile([128, 128, MC], bf, tag="wx")
                    nc.vector.tensor_tensor(
                        out=wx[:, :, :cols], in0=identx[:, :, :cols],
                        in1=exps[:, None, :cols]
                            .to_broadcast([128, 128, cols]),
                        op=mybir.AluOpType.mult,
                    )
                    par = ci % nps
                    for jj in range(cols):
                        nc.tensor.matmul(
                            out=pss[par][:],
                            lhsT=wx[:, :, jj],
                            rhs=g_t[:, jj, WD : WD + DOCW],
                            start=first[par] and jj == 0,
                            stop=(ci == last_ci[par] and jj == cols - 1),
                        )
                    first[par] = False

                osb = per_blk.tile([128, DOCW], f32, tag="osb",
                                   name=f"osb_{s}")
                nc.scalar.copy(out=osb[:], in_=pss[0][:])
                for par in range(1, nps):
                    # only one non-scalar PSUM input allowed per instruction
                    nc.vector.tensor_tensor(
                        out=osb[:], in0=osb[:],
                        in1=pss[par][:], op=mybir.AluOpType.add,
                    )
                nc.sync.dma_start(
                    out=doc_d[s * 128 : (s + 1) * 128, :], in_=osb[:],
                )

    nc.compile()
    return nc


# --------------------------------------------------------------------------
# entry point
# --------------------------------------------------------------------------

def _sig(plan):
    return tuple(tuple(int(x) for x in sl["BW"]) for sl in plan["slots"])


def kernel(word_embs, wordmem, bankmem, idx, lengths, _trace=False, **_kw):
    we = np.asarray(word_embs, dtype=np.float32)
    wm = np.asarray(wordmem, dtype=np.float32)
    bm = np.asarray(bankmem, dtype=np.float32)

    plan = _plan(idx, lengths)
    sig = _sig(plan)
    if _CACHE.get("sig") != sig:
        _CACHE["nc"] = _build_nc(plan)
        _CACHE["sig"] = sig
    nc = _CACHE["nc"]

    # host-normalized, bf16 fused table: [w-hat 128 | bank 400 | pad]
    wnorm = np.sqrt((wm * wm).sum(axis=1, dtype=np.float32))
    wn = wm / np.maximum(wnorm, np.float32(1e-12))[:, None]
    tb2 = np.zeros((V, FDP), dtype=BF16)
    tb2[:, :WD] = wn.astype(BF16)
    tb2[:, WD : WD + HD] = bm.astype(BF16)
    tb2[:, WD + HD] = BF16(1.0)  # denominator column: psum[:, HD] = sum(exps)
    qnorm = np.sqrt((we * we).sum(axis=1, dtype=np.float32))
    we_hat16 = (we / np.maximum(qnorm, np.float32(1e-12))[:, None]).astype(BF16)

    in_maps = []
    for c in range(N_CORES):
        m = _per_core_arrays(plan, we_hat16, c)
        m["tb"] = tb2
        in_maps.append(m)

    kw = {"trace": True, "trace_cores": [0]} if _trace else {}
    res = run_bass_kernel_spmd(nc, in_maps, core_ids=list(range(N_CORES)), **kw)
    if _trace:
        print(f"HW exec time: {res.exec_time_ns} ns")
        _CACHE["last_trace"] = res

    acc = np.zeros((NUM, HD), dtype=np.float64)
    den = np.zeros((NUM,), dtype=np.float64)
    for c in range(N_CORES):
        doc = res.results[c]["doc"]
        for s, sl in enumerate(plan["slots"]):
            ranks = sl["ranks"][c]
            rows = doc[s * 128 : (s + 1) * 128, :]
            for p, r in enumerate(ranks):
                t, _ = plan["pieces"][r]
                if t < 0:
                    continue
                acc[t] += rows[p, :HD]
                den[t] += rows[p, HD]
    out = (acc / np.maximum(den, 1e-30)[:, None]).astype(np.float32)
    return out

